# revision 1
# baseline (speedup 1.0000x reference)
"""Trainium2 Bass kernel for nn_BilinearSampler (triplane bilinear sampling).

Strategy (batch-parallel over 8 NeuronCores, one batch element per core):

Host prep (numpy, cheap integer/elementwise work on 0.8 MB of points):
  * For each plane, transpose features to [H, W, C] and build an
    "interleaved pair-row" table IL[y*128+x] = [c[y,x,:], c[y+1,x,:]]
    ([16384, 256] f32).  A query's 4 bilinear corners are then IL rows
    j, j+1 (j = y0*128+x0) = one contiguous 2 KB window.
  * Per query: cell index j (int16, SWDGE wrapped-16 layout) and the 4
    lerp scalars wx, 1-wx, wy, 1-wy in a queries-on-partitions layout
    (query n -> partition n%128, column n//128), f32 arithmetic matching
    the reference bit-for-bit.

Device kernel per core, per plane, per 1024-query chunk:
  * one SWDGE dma_gather: 1024 descriptors x 2 KB (measured ~260 GB/s;
    1 KB descriptors only sustain ~90 GB/s, which is why the pair-row
    interleave exists) -> G[128, 8, 512] with query on partition.
  * combine on VectorE with per-partition scalars, per 128-query group:
      H = G[:,g,256:512] * wx          (tensor_scalar,        FD 256)
      T = G[:,g,0:256] * (1-wx) + H    (scalar_tensor_tensor, FD 256)
      Y = T[:,128:256] * wy            (tensor_scalar,        FD 128)
      R = T[:,0:128]  * (1-wy) + Y     (scalar_tensor_tensor, FD 128)
  * chunk result [128, 8, 128] DMA'd to out[n, plane*128 : +128] rows.

gpsimd ap_gather was measured at ~28 ns/index (non-pipelined Q7 read
commands) and is not competitive; SWDGE dma_gather at >=2 KB/descriptor
is the fastest gather primitive on this part.
"""

import sys

sys.path.insert(0, "/opt/trn_rl_repo")

import numpy as np

B, N, C, R = 8, 32768, 128, 128
N_CORES = 8
CHUNK = 1024  # queries per dma_gather (>=2048 hangs the SWDGE path)
PAD_EPS = np.float32(1e-3)
CLIP_HI = np.float32(1.0 - 1e-5)

_PLANES = (("xz", 0, 2), ("xy", 0, 1), ("yz", 1, 2))  # (name, x_dim, y_dim)

_cache = {}


# --------------------------------------------------------------------------
# host-side prep
# --------------------------------------------------------------------------

def _coords(p_b):
    """p_b [N,3] f32 -> per-dim (floor int32, frac f32), f32 ops matching jax."""
    one = np.float32(1.0)
    uv = p_b / (one + np.float32(0.0) + PAD_EPS) + np.float32(0.5)
    uv = np.clip(uv, np.float32(0.0), CLIP_HI)
    x = uv * np.float32(R - 1)
    x0f = np.floor(x)
    frac = x - x0f
    x0 = np.clip(x0f, 0, R - 1).astype(np.int32)
    return x0, frac.astype(np.float32)


def _wrapped_idx(idx):
    """[N] int -> [128, N/16] int16 wrapped-16 replicated layout."""
    t = idx.reshape(N // 16, 16).T.astype(np.int16)
    return np.tile(t, (8, 1))


def _qpart(a):
    """[N] f32 -> [128, N/128] with query n at (n%128, n//128)."""
    return np.ascontiguousarray(a.reshape(N // 128, 128).T)


def _interleave(plane):
    """plane [C,R,R] f32 -> IL [R*R, 2*C] f32: row y*R+x = [ct[y,x,:], ct[y+1,x,:]]."""
    ct = np.transpose(plane, (1, 2, 0))  # [H=y, W=x, C]
    il = np.empty((R, R, 2, C), dtype=np.float32)
    il[:, :, 0, :] = ct
    il[:-1, :, 1, :] = ct[1:]
    il[-1, :, 1, :] = ct[-1]  # y=127 second halves are never gathered
    return il.reshape(R * R, 2 * C)


def _host_prep(p, c_xz, c_xy, c_yz):
    planes = (c_xz, c_xy, c_yz)
    in_maps = []
    for b in range(B):
        m = {}
        x0s, fracs = [], []
        for d in range(3):
            x0, fr = _coords(np.ascontiguousarray(p[b, :, d]))
            x0s.append(x0)
            fracs.append(fr)
        wcols = []
        for pl, (_, xd, yd) in enumerate(_PLANES):
            j = x0s[yd] * R + x0s[xd]
            m[f"idx{pl}"] = _wrapped_idx(j)
            wx, wy = fracs[xd], fracs[yd]
            wcols += [
                _qpart(wx),
                _qpart(np.float32(1.0) - wx),
                _qpart(wy),
                _qpart(np.float32(1.0) - wy),
            ]
            m[f"il{pl}"] = _interleave(planes[pl][b])
        m["w"] = np.concatenate(wcols, axis=1)  # [128, 12*(N/128)]
        in_maps.append(m)
    return in_maps


# --------------------------------------------------------------------------
# device program
# --------------------------------------------------------------------------

def _build_nc(reps=1):
    from contextlib import ExitStack

    import concourse.tile as tile
    from concourse import bacc, mybir
    from concourse.ap import AP

    FP32 = mybir.dt.float32
    I16 = mybir.dt.int16
    MULT = mybir.AluOpType.mult
    ADD = mybir.AluOpType.add

    ncols = N // 128  # weight columns per plane-quantity
    nchunks = N // CHUNK
    ng = CHUNK // 128

    nc = bacc.Bacc(
        "TRN2", target_bir_lowering=False, debug=False, num_devices=N_CORES
    )
    il_t = [
        nc.dram_tensor(f"il{pl}", [R * R, 2 * C], FP32, kind="ExternalInput")
        for pl in range(3)
    ]
    idx_t = [
        nc.dram_tensor(f"idx{pl}", [128, N // 16], I16, kind="ExternalInput").ap()
        for pl in range(3)
    ]
    w_t = nc.dram_tensor("w", [128, 12 * ncols], FP32, kind="ExternalInput").ap()
    out_t = nc.dram_tensor("out", [N, 3 * C], FP32, kind="ExternalOutput")

    with tile.TileContext(nc) as tc, ExitStack() as ctx:
        wp = ctx.enter_context(tc.tile_pool(name="wp", bufs=1))
        gp = ctx.enter_context(tc.tile_pool(name="gp", bufs=3))
        sp = ctx.enter_context(tc.tile_pool(name="sp", bufs=2))
        rp = ctx.enter_context(tc.tile_pool(name="rp", bufs=2))

        for _ in range(reps):
            tw = wp.tile([128, 12 * ncols], FP32, name="tw", tag="tw")
            nc.sync.dma_start(tw[:], w_t[:])
            tidx = []
            for pl in range(3):
                ti = wp.tile([128, N // 16], I16, name=f"ti{pl}", tag=f"ti{pl}")
                nc.sync.dma_start(ti[:], idx_t[pl][:])
                tidx.append(ti)

            for pl in range(3):
                in_ap = AP(il_t[pl], 0, [(2 * C, R * R - 1), (1, 4 * C)])
                for ch in range(nchunks):
                    tg = gp.tile([128, ng, 4 * C], FP32, name="tg", tag="tg")
                    nc.gpsimd.dma_gather(
                        tg[:],
                        in_ap,
                        tidx[pl][:, ch * (CHUNK // 16) : (ch + 1) * (CHUNK // 16)],
                        CHUNK,
                        CHUNK,
                        4 * C,
                        elem_step=2 * C,
                    )
                    res = rp.tile([128, ng, C], FP32, name="res", tag="res")
                    for g in range(ng):
                        col = ch * ng + g
                        wx = tw[:, (pl * 4 + 0) * ncols + col : (pl * 4 + 0) * ncols + col + 1]
                        mwx = tw[:, (pl * 4 + 1) * ncols + col : (pl * 4 + 1) * ncols + col + 1]
                        wy = tw[:, (pl * 4 + 2) * ncols + col : (pl * 4 + 2) * ncols + col + 1]
                        mwy = tw[:, (pl * 4 + 3) * ncols + col : (pl * 4 + 3) * ncols + col + 1]
                        th = sp.tile([128, 2 * C], FP32, name="th", tag="th")
                        nc.vector.tensor_scalar(th[:], tg[:, g, 2 * C : 4 * C], wx, None, MULT)
                        tt = sp.tile([128, 2 * C], FP32, name="tt", tag="tt")
                        nc.vector.scalar_tensor_tensor(
                            tt[:], tg[:, g, 0 : 2 * C], mwx, th[:], MULT, ADD
                        )
                        ty = sp.tile([128, C], FP32, name="ty", tag="ty")
                        nc.vector.tensor_scalar(ty[:], tt[:, C : 2 * C], wy, None, MULT)
                        nc.vector.scalar_tensor_tensor(
                            res[:, g, :], tt[:, 0:C], mwy, ty[:], MULT, ADD
                        )
                    # out rows n = ch*CHUNK + g*128 + p, columns [pl*C, (pl+1)*C)
                    dst = AP(
                        out_t,
                        ch * CHUNK * 3 * C + pl * C,
                        [(3 * C, 128), (128 * 3 * C, ng), (1, C)],
                    )
                    nc.sync.dma_start(dst, res[:])
    nc.compile()
    return nc


# --------------------------------------------------------------------------
# jit-once PJRT runner (axon)
# --------------------------------------------------------------------------

class _Runner:
    def __init__(self, nc, n_cores=N_CORES):
        import jax
        from jax.experimental.shard_map import shard_map
        from jax.sharding import Mesh, PartitionSpec

        import concourse.mybir as mybir
        from concourse.bass2jax import (
            _bass_exec_p,
            install_neuronx_cc_hook,
            partition_id_tensor,
        )

        install_neuronx_cc_hook()
        self.jax = jax
        self.n_cores = n_cores
        pname = nc.partition_id_tensor.name if nc.partition_id_tensor else None

        in_names, out_names, out_avals, zero_outs = [], [], [], []
        for alloc in nc.m.functions[0].allocations:
            if not isinstance(alloc, mybir.MemoryLocationSet):
                continue
            name = alloc.memorylocations[0].name
            if alloc.kind == "ExternalInput":
                if name != pname:
                    in_names.append(name)
            elif alloc.kind == "ExternalOutput":
                shape = tuple(alloc.tensor_shape)
                dtype = mybir.dt.np(alloc.dtype)
                out_names.append(name)
                out_avals.append(jax.core.ShapedArray(shape, dtype))
                zero_outs.append(np.zeros(shape, dtype))
        n_params = len(in_names)
        all_in = list(in_names) + list(out_names)
        if pname is not None:
            all_in.append(pname)
        self.in_names, self.out_names, self.out_avals = in_names, out_names, out_avals
        self.n_params = n_params

        def _body(*args):
            ops = list(args)
            if pname is not None:
                ops.append(partition_id_tensor())
            return tuple(
                _bass_exec_p.bind(
                    *ops,
                    out_avals=tuple(out_avals),
                    in_names=tuple(all_in),
                    out_names=tuple(out_names),
                    lowering_input_output_aliases=(),
                    sim_require_finite=True,
                    sim_require_nnan=True,
                    nc=nc,
                )
            )

        devices = jax.devices()[:n_cores]
        mesh = Mesh(np.asarray(devices), ("core",))
        specs = (PartitionSpec("core"),)
        self.fn = jax.jit(
            shard_map(
                _body,
                mesh=mesh,
                in_specs=specs * (n_params + len(out_names)),
                out_specs=specs * len(out_names),
                check_rep=False,
            ),
            keep_unused=True,
        )
        self._zeros = [
            jax.device_put(np.zeros((n_cores * z.shape[0], *z.shape[1:]), z.dtype))
            for z in zero_outs
        ]

    def prepare(self, in_maps):
        concat = [
            np.concatenate([np.asarray(m[name]) for m in in_maps], axis=0)
            for name in self.in_names
        ]
        return [self.jax.device_put(a) for a in concat] + self._zeros

    def run_prepared(self, args):
        outs = self.fn(*args)
        self.jax.block_until_ready(outs)
        return outs

    def collect(self, outs):
        return [
            {
                name: np.asarray(outs[i]).reshape(
                    self.n_cores, *self.out_avals[i].shape
                )[c]
                for i, name in enumerate(self.out_names)
            }
            for c in range(self.n_cores)
        ]


def _get_runner(reps=1):
    key = ("runner", reps)
    if key not in _cache:
        _cache[key] = _Runner(_build_nc(reps=reps))
    return _cache[key]


# --------------------------------------------------------------------------
# entry point
# --------------------------------------------------------------------------

def kernel(p, c_xz, c_xy, c_yz):
    p = np.asarray(p, dtype=np.float32)
    c_xz = np.asarray(c_xz, dtype=np.float32)
    c_xy = np.asarray(c_xy, dtype=np.float32)
    c_yz = np.asarray(c_yz, dtype=np.float32)
    in_maps = _host_prep(p, c_xz, c_xy, c_yz)
    r = _get_runner()
    outs = r.collect(r.run_prepared(r.prepare(in_maps)))
    return np.stack([outs[b]["out"] for b in range(B)], axis=0)


# revision 4
# speedup vs baseline: 2.7040x; 2.7040x over previous
"""Trainium2 Bass kernel for nn_BilinearSampler (triplane bilinear sampling).

Strategy (batch-parallel over 8 NeuronCores, one batch element per core):

Host prep (numpy, cheap integer/elementwise work on 0.8 MB of points):
  * For each plane, transpose features to [H, W, C] and build an
    "interleaved pair-row" table IL[y*128+x] = [c[y,x,:], c[y+1,x,:]]
    ([16384, 256] f32).  A query's 4 bilinear corners are then IL rows
    j, j+1 (j = y0*128+x0) = one contiguous 2 KB window.
  * Per query: cell index j (int16, SWDGE wrapped-16 layout) and the 4
    lerp scalars wx, 1-wx, wy, 1-wy in a queries-on-partitions layout
    (query n -> partition n%128, column n//128), f32 arithmetic matching
    the reference bit-for-bit.

Device kernel per core, per plane, per 1024-query chunk:
  * one SWDGE dma_gather: 1024 descriptors x 2 KB (measured ~260 GB/s;
    1 KB descriptors only sustain ~90 GB/s, which is why the pair-row
    interleave exists) -> G[128, 8, 512] with query on partition.
  * combine on VectorE with per-partition scalars, per 128-query group:
      H = G[:,g,256:512] * wx          (tensor_scalar,        FD 256)
      T = G[:,g,0:256] * (1-wx) + H    (scalar_tensor_tensor, FD 256)
      Y = T[:,128:256] * wy            (tensor_scalar,        FD 128)
      R = T[:,0:128]  * (1-wy) + Y     (scalar_tensor_tensor, FD 128)
  * chunk result [128, 8, 128] DMA'd to out[n, plane*128 : +128] rows.

gpsimd ap_gather was measured at ~28 ns/index (non-pipelined Q7 read
commands) and is not competitive; SWDGE dma_gather at >=2 KB/descriptor
is the fastest gather primitive on this part.
"""

import sys

sys.path.insert(0, "/opt/trn_rl_repo")

import numpy as np

B, N, C, R = 8, 32768, 128, 128
N_CORES = 8
CHUNK = 1024  # queries per dma_gather (>=2048 hangs the SWDGE path)
PAD_EPS = np.float32(1e-3)
CLIP_HI = np.float32(1.0 - 1e-5)

_PLANES = (("xz", 0, 2), ("xy", 0, 1), ("yz", 1, 2))  # (name, x_dim, y_dim)

_cache = {}


def _register_lerp2():
    """Register a custom DVE op: out = Src0*C0 + Src1*C1 (per-partition
    scalars) — one full lerp per instruction instead of a
    tensor_scalar + scalar_tensor_tensor pair."""
    from concourse import dve_ops
    from concourse.dve_spec import C0, C1, Spec, Src0, Src1, _has_src1, lower
    from concourse.dve_uop import DveOpSpec

    name = "LERP2_ANT"
    for o in dve_ops.OPS:
        if o.name == name:
            return o
    spec = Spec(
        body=Src0 * C0 + Src1 * C1,
        reference=lambda in0, in1, s0, s1, imm2: in0.astype(np.float32) * s0
        + in1.astype(np.float32) * s1,
    )
    row = dve_ops._CUSTOM_DVE_ROW_BASE + len(dve_ops.OPS)
    assert row < 0x20
    shas = {}
    for ver in ("v3", "v4"):
        s_ = DveOpSpec(name=name, opcode=row, uops=lower(spec, ver=ver), rd1_en=_has_src1(spec))
        shas[ver] = s_.sha(ver)
    op = dve_ops.DveOp(name, spec, subdim=False, uops_sha=shas)
    dve_ops.OPS.append(op)
    dve_ops.CUSTOM_DVE_SPECS[name] = spec
    dve_ops._SUB_OPCODE_FOR_NAME[name] = row
    return op


# --------------------------------------------------------------------------
# host-side prep
# --------------------------------------------------------------------------

def _coords(p_b):
    """p_b [N,3] f32 -> per-dim (floor int32, frac f32), f32 ops matching jax."""
    one = np.float32(1.0)
    uv = p_b / (one + np.float32(0.0) + PAD_EPS) + np.float32(0.5)
    uv = np.clip(uv, np.float32(0.0), CLIP_HI)
    x = uv * np.float32(R - 1)
    x0f = np.floor(x)
    frac = x - x0f
    x0 = np.clip(x0f, 0, R - 1).astype(np.int32)
    return x0, frac.astype(np.float32)


def _wrapped_idx(idx):
    """[N] int -> [128, N/16] int16 wrapped-16 replicated layout."""
    t = idx.reshape(N // 16, 16).T.astype(np.int16)
    return np.tile(t, (8, 1))


def _qpart(a):
    """[N] f32 -> [128, N/128] with query n at (n%128, n//128)."""
    return np.ascontiguousarray(a.reshape(N // 128, 128).T)


def _interleave(plane):
    """plane [C,R,R] f32 -> IL [R*R, 2*C] f32: row y*R+x = [ct[y,x,:], ct[y+1,x,:]]."""
    ct = np.transpose(plane, (1, 2, 0))  # [H=y, W=x, C]
    il = np.empty((R, R, 2, C), dtype=np.float32)
    il[:, :, 0, :] = ct
    il[:-1, :, 1, :] = ct[1:]
    il[-1, :, 1, :] = ct[-1]  # y=127 second halves are never gathered
    return il.reshape(R * R, 2 * C)


def _host_prep(p, c_xz, c_xy, c_yz):
    planes = (c_xz, c_xy, c_yz)
    in_maps = []
    for b in range(B):
        m = {}
        x0s, fracs = [], []
        for d in range(3):
            x0, fr = _coords(np.ascontiguousarray(p[b, :, d]))
            x0s.append(x0)
            fracs.append(fr)
        wcols = []
        for pl, (_, xd, yd) in enumerate(_PLANES):
            j = x0s[yd] * R + x0s[xd]
            m[f"idx{pl}"] = _wrapped_idx(j)
            wx, wy = fracs[xd], fracs[yd]
            wcols += [
                _qpart(wx),
                _qpart(np.float32(1.0) - wx),
                _qpart(wy),
                _qpart(np.float32(1.0) - wy),
            ]
            m[f"il{pl}"] = _interleave(planes[pl][b])
        m["w"] = np.concatenate(wcols, axis=1)  # [128, 12*(N/128)]
        in_maps.append(m)
    return in_maps


# --------------------------------------------------------------------------
# device program
# --------------------------------------------------------------------------

def _build_nc(reps=1, probe="full"):
    import os
    from contextlib import ExitStack

    import concourse.tile as tile
    from concourse import bacc, mybir
    from concourse.ap import AP

    FP32 = mybir.dt.float32
    I16 = mybir.dt.int16
    MULT = mybir.AluOpType.mult
    ADD = mybir.AluOpType.add
    lerp2 = _register_lerp2()

    ncols = N // 128  # weight columns per plane-quantity
    nchunks = N // CHUNK
    ng = CHUNK // 128

    nc = bacc.Bacc(
        "TRN2", target_bir_lowering=False, debug=False, num_devices=N_CORES
    )
    il_t = [
        nc.dram_tensor(f"il{pl}", [R * R, 2 * C], FP32, kind="ExternalInput")
        for pl in range(3)
    ]
    idx_t = [
        nc.dram_tensor(f"idx{pl}", [128, N // 16], I16, kind="ExternalInput").ap()
        for pl in range(3)
    ]
    w_t = nc.dram_tensor("w", [128, 12 * ncols], FP32, kind="ExternalInput").ap()
    out_t = nc.dram_tensor("out", [N, 3 * C], FP32, kind="ExternalOutput")

    with tile.TileContext(nc) as tc, ExitStack() as ctx:
        wp = ctx.enter_context(tc.tile_pool(name="wp", bufs=1))
        gp = ctx.enter_context(tc.tile_pool(name="gp", bufs=3))
        sp = ctx.enter_context(tc.tile_pool(name="sp", bufs=2))
        rp = ctx.enter_context(tc.tile_pool(name="rp", bufs=2))

        for _ in range(reps):
            tw = wp.tile([128, 12 * ncols], FP32, name="tw", tag="tw")
            nc.sync.dma_start(tw[:], w_t[:])
            tidx = []
            for pl in range(3):
                ti = wp.tile([128, N // 16], I16, name=f"ti{pl}", tag=f"ti{pl}")
                nc.sync.dma_start(ti[:], idx_t[pl][:])
                tidx.append(ti)

            do_gather = probe in ("full", "gather")
            do_combine = probe in ("full", "combine")
            tg_fixed = None
            if not do_gather:
                tg_fixed = gp.tile([128, ng, 4 * C], FP32, name="tgf", tag="tg")
                nc.vector.memset(tg_fixed[:], 1.0)
            for pl in range(3):
                in_ap = AP(il_t[pl], 0, [(2 * C, R * R - 1), (1, 4 * C)])
                for ch in range(nchunks):
                    if do_gather:
                        tg = gp.tile([128, ng, 4 * C], FP32, name="tg", tag="tg")
                        nc.gpsimd.dma_gather(
                        tg[:],
                        in_ap,
                        tidx[pl][:, ch * (CHUNK // 16) : (ch + 1) * (CHUNK // 16)],
                        CHUNK,
                        CHUNK,
                        4 * C,
                        elem_step=2 * C,
                        )
                    else:
                        tg = tg_fixed
                    if not do_combine:
                        continue
                    res = rp.tile([128, ng, C], FP32, name="res", tag="res")
                    for g in range(ng):
                        col = ch * ng + g
                        wx = tw[:, (pl * 4 + 0) * ncols + col : (pl * 4 + 0) * ncols + col + 1]
                        mwx = tw[:, (pl * 4 + 1) * ncols + col : (pl * 4 + 1) * ncols + col + 1]
                        wy = tw[:, (pl * 4 + 2) * ncols + col : (pl * 4 + 2) * ncols + col + 1]
                        mwy = tw[:, (pl * 4 + 3) * ncols + col : (pl * 4 + 3) * ncols + col + 1]
                        tt = sp.tile([128, 2 * C], FP32, name="tt", tag="tt")
                        nc.vector._custom_dve(
                            lerp2, out=tt[:], in0=tg[:, g, 0 : 2 * C],
                            in1=tg[:, g, 2 * C : 4 * C], s0=mwx, s1=wx,
                        )
                        nc.vector._custom_dve(
                            lerp2, out=res[:, g, :], in0=tt[:, 0:C],
                            in1=tt[:, C : 2 * C], s0=mwy, s1=wy,
                        )
                    # out rows n = ch*CHUNK + g*128 + p, columns [pl*C, (pl+1)*C)
                    dst = AP(
                        out_t,
                        ch * CHUNK * 3 * C + pl * C,
                        [(3 * C, 128), (128 * 3 * C, ng), (1, C)],
                    )
                    nc.sync.dma_start(dst, res[:])
    nc.compile()
    return nc


# --------------------------------------------------------------------------
# jit-once PJRT runner (axon)
# --------------------------------------------------------------------------

class _Runner:
    def __init__(self, nc, n_cores=N_CORES):
        import jax
        from jax.experimental.shard_map import shard_map
        from jax.sharding import Mesh, PartitionSpec

        import concourse.mybir as mybir
        from concourse.bass2jax import (
            _bass_exec_p,
            install_neuronx_cc_hook,
            partition_id_tensor,
        )

        install_neuronx_cc_hook()
        self.jax = jax
        self.n_cores = n_cores
        pname = nc.partition_id_tensor.name if nc.partition_id_tensor else None

        in_names, out_names, out_avals, zero_outs = [], [], [], []
        for alloc in nc.m.functions[0].allocations:
            if not isinstance(alloc, mybir.MemoryLocationSet):
                continue
            name = alloc.memorylocations[0].name
            if alloc.kind == "ExternalInput":
                if name != pname:
                    in_names.append(name)
            elif alloc.kind == "ExternalOutput":
                shape = tuple(alloc.tensor_shape)
                dtype = mybir.dt.np(alloc.dtype)
                out_names.append(name)
                out_avals.append(jax.core.ShapedArray(shape, dtype))
                zero_outs.append(np.zeros(shape, dtype))
        n_params = len(in_names)
        all_in = list(in_names) + list(out_names)
        if pname is not None:
            all_in.append(pname)
        self.in_names, self.out_names, self.out_avals = in_names, out_names, out_avals
        self.n_params = n_params

        def _body(*args):
            ops = list(args)
            if pname is not None:
                ops.append(partition_id_tensor())
            return tuple(
                _bass_exec_p.bind(
                    *ops,
                    out_avals=tuple(out_avals),
                    in_names=tuple(all_in),
                    out_names=tuple(out_names),
                    lowering_input_output_aliases=(),
                    sim_require_finite=True,
                    sim_require_nnan=True,
                    nc=nc,
                )
            )

        devices = jax.devices()[:n_cores]
        mesh = Mesh(np.asarray(devices), ("core",))
        specs = (PartitionSpec("core"),)
        self.fn = jax.jit(
            shard_map(
                _body,
                mesh=mesh,
                in_specs=specs * (n_params + len(out_names)),
                out_specs=specs * len(out_names),
                check_rep=False,
            ),
            keep_unused=True,
        )
        self._zeros = [
            jax.device_put(np.zeros((n_cores * z.shape[0], *z.shape[1:]), z.dtype))
            for z in zero_outs
        ]

    def prepare(self, in_maps):
        concat = [
            np.concatenate([np.asarray(m[name]) for m in in_maps], axis=0)
            for name in self.in_names
        ]
        return [self.jax.device_put(a) for a in concat] + self._zeros

    def run_prepared(self, args):
        outs = self.fn(*args)
        self.jax.block_until_ready(outs)
        return outs

    def collect(self, outs):
        return [
            {
                name: np.asarray(outs[i]).reshape(
                    self.n_cores, *self.out_avals[i].shape
                )[c]
                for i, name in enumerate(self.out_names)
            }
            for c in range(self.n_cores)
        ]


def _get_runner(reps=1):
    key = ("runner", reps)
    if key not in _cache:
        _cache[key] = _Runner(_build_nc(reps=reps))
    return _cache[key]


# --------------------------------------------------------------------------
# entry point
# --------------------------------------------------------------------------

def kernel(p, c_xz, c_xy, c_yz):
    p = np.asarray(p, dtype=np.float32)
    c_xz = np.asarray(c_xz, dtype=np.float32)
    c_xy = np.asarray(c_xy, dtype=np.float32)
    c_yz = np.asarray(c_yz, dtype=np.float32)
    in_maps = _host_prep(p, c_xz, c_xy, c_yz)
    r = _get_runner()
    outs = r.collect(r.run_prepared(r.prepare(in_maps)))
    return np.stack([outs[b]["out"] for b in range(B)], axis=0)
